# revision 34
# baseline (speedup 1.0000x reference)
"""MQA attention kernel for Trainium2, 8-core SPMD (v11).

Problem: Q [2, 8, 2048, 64] fp32, K/V [2, 1, 2048, 64] fp32 (shared head).
out[b,h,q,:] = softmax(Q[b,h,q,:] @ K[b,0]^T / 8) @ V[b,0].

Sharding: 16 (b,h) pairs over 8 cores -> core c handles b = c//4,
heads 2*(c%4), 2*(c%4)+1 (both heads share one K/V slice).

Design notes:
  - "Permuted-s" staging: inputs land as Xh[p, c, ...] = X[s=16p+c]
    (contiguous 4KB descriptors; softmax doesn't care about key/query
    order, V uses the same key order; only the output DMA un-permutes).
  - All staged tensors are split into per-chunk TILES (Tile tracks
    dependencies at whole-tile granularity, so chunked tiles are what
    lets the main loop start as soon as chunk 0 is ready).
  - Critical path (K + Q chunk 0) via HWDGE + ACT/DVE fp16 casts; V and
    Q chunks 1-3 stream via SWDGE cast-DMAs (Q gated behind the
    critical-path cast so they can't starve it).
  - QT/KT via merged SBUF->SBUF XBAR transpose-DMAs (3D-out = chunked
    128-partition transposes; K is cast column-duplicated so K^T lands
    replicated on both partition halves).
  - ONE flat 64-step software pipeline over (qb, kt): MM1 two concurrent
    row-group matmuls (contract=64) -> per-head single-bank score tiles
    (3-deep rotation); exp split ACT (true, h0) / DVE (Schraudolph
    int16-bitcast, h1); MM2 (V_aug = [V | ones | 0-pad], col 64 =
    denominator) trails by 4 steps, crossing qb boundaries seamlessly.
    PSUM: 6 score banks + 2 out banks.
  - Drain per (qb, h): PSUM->SBUF fp16 copy (ACT h0 / DVE h1), merged
    XBAR transpose [80,512]->[128,4,80] (h0 Sync ring, h1 Scalar ring,
    concurrent), DVE reciprocal + normalize, un-permuting DMA out
    (final-qb DMAs split across rings). One stage per pipeline step.
  - Two PE warmup phases keep HAM at K=8/8 through the staging window.
"""

import numpy as np

import concourse.bass as bass
import concourse.bacc as bacc
import concourse.mybir as mybir
import concourse.tile as tile
from concourse.bass_utils import run_bass_kernel_spmd

F32 = mybir.dt.float32
F16 = mybir.dt.float16
I16 = mybir.dt.int16

B, H, S, D = 2, 8, 2048, 64
HPC = 2            # heads per core
NCORES = 8
QB = 512           # query block (PSUM bank free-dim limit for fp32)
NQB = S // QB      # 4
KT_TILE = 128      # keys per k-tile (PE contract partition limit)
NKT = S // KT_TILE # 16
NC = NKT // NQB    # 4 c-chunks per qb block
MO = 80            # drained rows: 64 V + 1 denom + 15 pad (16 | 80)
VW = 128           # V_aug weight cols
NSTEP = NQB * NKT  # 64 flat pipeline steps
SCALE = 1.0 / np.sqrt(np.float32(D))  # 0.125
C_SHIFT = 2.0      # exp(z - C_SHIFT): bounds p in fp16/int16; cancels in softmax

# Schraudolph fp16 exp: i16 = round(score*A2 + B2); bitcast fp16 ~= exp(score/8 - C)
A_FP16 = 1024.0 / np.log(2.0)              # 2^10 / ln 2
A2 = float(SCALE) * A_FP16                 # folded score scale
B2 = 15.0 * 1024.0 - C_SHIFT * A_FP16      # exponent bias - shift
SKEW = 3           # MM2 trails MM1 by SKEW+1 steps (check precedes append)


def build_nc():
    nc = bacc.Bacc(None)
    Qd = nc.declare_dram_parameter("q", [HPC, S, D], F32, isOutput=False)
    Kd = nc.declare_dram_parameter("k", [S, D], F32, isOutput=False)
    Vd = nc.declare_dram_parameter("v", [S, D], F32, isOutput=False)
    Od = nc.declare_dram_parameter("o", [HPC, S, D], F32, isOutput=True)

    with tile.TileContext(nc) as tc:
        with (
            tc.tile_pool(name="const", bufs=1) as constp,
            tc.tile_pool(name="stage", bufs=1) as stp,
            tc.tile_pool(name="qk", bufs=1) as qkp,
            tc.tile_pool(name="vt", bufs=1) as vp,
            tc.tile_pool(name="pt", bufs=6) as ptp,
            tc.tile_pool(name="otF", bufs=2) as otFp,
            tc.tile_pool(name="otT", bufs=4) as otTp,
            tc.tile_pool(name="rec", bufs=4) as recp,
            tc.tile_pool(name="outsb", bufs=4) as outp,
            tc.tile_pool(name="psS", bufs=3, space="PSUM") as psSp,
            tc.tile_pool(name="psO", bufs=1, space="PSUM") as psOp,
        ):
            # Prime the exp table load so the ~2.7us ACT_TABLE_LOAD overlaps
            # the input DMA phase instead of stalling the first real exp.
            dummy = constp.tile([128, 64], F16)
            nc.vector.memset(dummy[:], 0.0)
            dummy32 = constp.tile([128, 16], F32)
            nc.vector.memset(dummy32[:], 0.0)
            nc.scalar.activation(
                dummy32[:], dummy32[:], mybir.ActivationFunctionType.Exp
            )

            # per-partition bias AP for the ACT exp (const-AP registry only
            # carries pre-registered values)
            bias_ap = constp.tile([128, 1], F32)
            nc.vector.memset(bias_ap[:], -float(C_SHIFT))

            # warmup output scribbles into the shared out accumulator
            # (cleared by the start=True MM2 later)
            ps_o = [psOp.tile([128, QB], F32, name=f"psO{h}") for h in range(HPC)]

            # PE warmup phase 1: unblocked, runs as soon as the PE is free.
            for _ in range(40):
                nc.tensor.matmul(
                    ps_o[0][0:64, 0:64],
                    lhsT=dummy[:, 0:64],
                    rhs=dummy[:],
                    start=True,
                    stop=True,
                )

            # ---- input staging (all per-c-chunk tiles) ----
            # SWDGE transfers execute in ring order, so emitting the critical
            # Q chunk 0 cast-DMA FIRST guarantees it cannot be starved by the
            # V / later-Q streams. K rides HWDGE (16-engine spread) alone.
            Qh = [
                stp.tile([128, NC, HPC, D], F16, name=f"Qh{c}") for c in range(NQB)
            ]
            for h in range(HPC):
                nc.gpsimd.dma_start(
                    out=Qh[0][:, :, h, :],
                    in_=Qd.ap()[h].rearrange("(p c) d -> p c d", p=128)[:, 0:NC, :],
                )
            Kn = stp.tile([128, NKT, D], F32, name="Kn")
            nc.scalar.dma_start(
                out=Kn[:], in_=Kd.ap().rearrange("(p c) d -> p c d", p=128)
            )

            # V_aug chunks [128k, NC, 128] fp16: cols 0-63 = V (same permuted
            # key order as KT), col 64 = 1.0 (denominator), cols 65-127 = 0.
            Vt = [vp.tile([128, NC, VW], F16, name=f"Vt{c}") for c in range(NQB)]
            for c in range(NQB):
                nc.vector.memset(Vt[c][:, :, D + 1 : VW], 0.0)
                nc.vector.memset(Vt[c][:, :, D : D + 1], 1.0)
                nc.gpsimd.dma_start(
                    out=Vt[c][:, :, 0:D],
                    in_=Vd.ap().rearrange("(p c) d -> p c d", p=128)[
                        :, c * NC : (c + 1) * NC, :
                    ],
                )
            for c in range(1, NQB):
                cs = slice(c * NC, (c + 1) * NC)
                for h in range(HPC):
                    nc.gpsimd.dma_start(
                        out=Qh[c][:, :, h, :],
                        in_=Qd.ap()[h].rearrange("(p c) d -> p c d", p=128)[
                            :, cs, :
                        ],
                    )

            # K cast duplicated into both 64-col halves (the XBAR consumes
            # 128-wide input column tiles; the duplicate lands K^T on both
            # partition halves of KT). Halves split across DVE and ACT.
            Kh = [
                stp.tile([128, NC, 2, D], F16, name=f"Kh{c}") for c in range(NQB)
            ]
            for c in range(NQB):
                cs = slice(c * NC, (c + 1) * NC)
                nc.vector.tensor_copy(Kh[c][:, :, 0, :], Kn[:, cs, :])
                nc.scalar.copy(Kh[c][:, :, 1, :], Kn[:, cs, :])

            # PE warmup phase 2: gated on the K load so it bridges the
            # staging window right up to the main loop.
            for _ in range(30):
                nc.tensor.matmul(
                    ps_o[0][0:16, 0:64],
                    lhsT=dummy32[:],
                    rhs=Kn[:, 0, :],
                    start=True,
                    stop=True,
                )

            # KT/QT chunks [128, NC, 128] fp16 via merged XBAR transposes
            # alternating between the two HWDGE rings (first K/Q pair runs
            # concurrently): col (c, p) is key/query s = 16p + c.
            KT = [
                qkp.tile([128, NC, 128], F16, name=f"KT{c}") for c in range(NQB)
            ]
            QT = [
                qkp.tile([128, NC, 128], F16, name=f"QT{c}") for c in range(NQB)
            ]
            for c in range(NQB):
                nc.sync.dma_start(
                    out=KT[c][:],
                    in_=Kh[c][:].rearrange("p c r d -> p (c r d)"),
                    transpose=True,
                )
                nc.scalar.dma_start(
                    out=QT[c][:],
                    in_=Qh[c][:].rearrange("p c h d -> p (c h d)"),
                    transpose=True,
                )

            def QTs(h, qb):
                # [64, 512] moving operand for head h, query block qb
                return QT[qb][64 * h : 64 * (h + 1), :, :].rearrange(
                    "p c s -> p (c s)"
                )

            def KTs(h, kt):
                return KT[kt // NC][64 * h : 64 * (h + 1), kt % NC, :]

            # ---- drains, emitted one stage per pipeline step ----
            def drain_stages(qb, ps_o):
                st = {"otF": [None, None], "otT": [None, None], "rec": [None, None],
                      "out": [None, None]}
                ring = [nc.sync, nc.scalar]  # per-head DMA ring
                last = qb == NQB - 1

                def s_copy(h):
                    def f():
                        st["otF"][h] = otFp.tile([MO, QB], F16, name=f"otF{h}")
                        if h == 0:
                            nc.scalar.copy(st["otF"][h][:], ps_o[h][0:MO, :])
                        else:
                            nc.vector.tensor_copy(st["otF"][h][:], ps_o[h][0:MO, :])
                    return f

                def s_transp(h):
                    def f():
                        st["otT"][h] = otTp.tile(
                            [128, NC, MO], F16, name=f"otT{h}"
                        )
                        ring[h].dma_start(
                            out=st["otT"][h][:], in_=st["otF"][h][:], transpose=True
                        )
                    return f

                def s_recip(h):
                    def f():
                        otT = st["otT"][h]
                        rec = recp.tile([128, NC, 1], F32)
                        nc.vector.reciprocal(rec[:], otT[:, :, D : D + 1])
                        st["rec"][h] = rec
                        st["out"][h] = outp.tile([128, NC, D], F32, name=f"osb{h}")
                        for j in range(NC // 2):
                            nc.vector.tensor_scalar_mul(
                                st["out"][h][:, j, :], otT[:, j, 0:D], rec[:, j, :]
                            )
                    return f

                def s_norm(h):
                    def f():
                        otT, rec, outsb = st["otT"][h], st["rec"][h], st["out"][h]
                        oap = Od.ap()[h].rearrange("(p c) d -> p c d", p=128)
                        for j in range(NC // 2, NC):
                            nc.vector.tensor_scalar_mul(
                                outsb[:, j, :], otT[:, j, 0:D], rec[:, j, :]
                            )
                        if last:
                            # tail: j-paired DMAs across both rings so the
                            # exposed final transfers overlap
                            for j2 in range(2):
                                ring[j2 ^ h].dma_start(
                                    out=oap[
                                        :, qb * NC + 2 * j2 : qb * NC + 2 * j2 + 2, :
                                    ],
                                    in_=outsb[:, 2 * j2 : 2 * j2 + 2, :],
                                )
                        else:
                            ring[h].dma_start(
                                out=oap[:, qb * NC : (qb + 1) * NC, :],
                                in_=outsb[:],
                            )
                    return f

                return [
                    s_copy(0), s_copy(1), s_transp(0), s_transp(1),
                    s_recip(0), s_norm(0), s_recip(1), s_norm(1),
                ]

            # ---- ONE flat software-pipelined loop over all 64 (qb, kt) ----
            pend = []           # (ps_o_pair, qb, kt, pt0, pt1) awaiting MM2
            pending_drain = []  # drain stages of the qb that just finished
            ps_o_cur = ps_o
            for s in range(NSTEP + SKEW + 1):
                if s < NSTEP:
                    qb, kt = divmod(s, NKT)
                    if kt == 0 and qb > 0:
                        ps_o_cur = [
                            psOp.tile([128, QB], F32, name=f"psO{h}")
                            for h in range(HPC)
                        ]
                    ps_s = [
                        psSp.tile([128, QB], F32, name=f"psS{h}")
                        for h in range(HPC)
                    ]
                    for h in range(HPC):
                        nc.tensor.matmul(
                            ps_s[h][:],
                            lhsT=KTs(h, kt),
                            rhs=QTs(h, qb),
                            start=True,
                            stop=True,
                        )
                if len(pend) > (SKEW if s < NSTEP else 0):
                    po, pqb, pkt, p0, p1 = pend.pop(0)
                    for h, rhs in ((0, p0[:]), (1, p1[:].bitcast(F16))):
                        nc.tensor.matmul(
                            po[h][:],
                            lhsT=Vt[pkt // NC][:, pkt % NC, :],
                            rhs=rhs,
                            start=(pkt == 0),
                            stop=(pkt == NKT - 1),
                        )
                    if pkt == NKT - 1:
                        assert not pending_drain
                        pending_drain = drain_stages(pqb, po)
                if pending_drain:
                    pending_drain.pop(0)()
                if s < NSTEP:
                    # exp: ACT (true) for h0, DVE (Schraudolph) for h1
                    pt0 = ptp.tile([128, QB], F16, name="pt0")
                    nc.scalar.activation(
                        pt0[:],
                        ps_s[0][:],
                        mybir.ActivationFunctionType.Exp,
                        scale=float(SCALE),
                        bias=bias_ap[:],
                    )
                    pt1 = ptp.tile([128, QB], I16, name="pt1")
                    nc.vector.tensor_scalar(
                        pt1[:],
                        ps_s[1][:],
                        float(A2),
                        float(B2),
                        op0=mybir.AluOpType.mult,
                        op1=mybir.AluOpType.add,
                    )
                    pend.append((ps_o_cur, qb, kt, pt0, pt1))
            assert not pend
            for f in pending_drain:
                f()
    nc.compile()
    return nc


_CACHED = {}


def _get_nc():
    if "nc" not in _CACHED:
        _CACHED["nc"] = build_nc()
    return _CACHED["nc"]


def _shard(Q, K, V):
    in_maps = []
    for c in range(NCORES):
        b = c // 4
        h0 = (c % 4) * HPC
        in_maps.append(
            {
                "q": np.ascontiguousarray(np.asarray(Q, np.float32)[b, h0 : h0 + HPC]),
                "k": np.ascontiguousarray(np.asarray(K, np.float32)[b, 0]),
                "v": np.ascontiguousarray(np.asarray(V, np.float32)[b, 0]),
            }
        )
    return in_maps


def kernel(Q, K, V, trace=False):
    nc = _get_nc()
    res = run_bass_kernel_spmd(nc, _shard(Q, K, V), list(range(NCORES)), trace=trace)
    _CACHED["last_result"] = res
    O = np.empty((B, H, S, D), np.float32)
    for c, r in enumerate(res.results):
        b = c // 4
        h0 = (c % 4) * HPC
        O[b, h0 : h0 + HPC] = r["o"]
    return O


# revision 38
# speedup vs baseline: 1.0776x; 1.0776x over previous
"""MQA attention kernel for Trainium2, 8-core SPMD (v11).

Problem: Q [2, 8, 2048, 64] fp32, K/V [2, 1, 2048, 64] fp32 (shared head).
out[b,h,q,:] = softmax(Q[b,h,q,:] @ K[b,0]^T / 8) @ V[b,0].

Sharding: 16 (b,h) pairs over 8 cores -> core c handles b = c//4,
heads 2*(c%4), 2*(c%4)+1 (both heads share one K/V slice).

Design notes:
  - "Permuted-s" staging: inputs land as Xh[p, c, ...] = X[s=16p+c]
    (contiguous 4KB descriptors; softmax doesn't care about key/query
    order, V uses the same key order; only the output DMA un-permutes).
  - All staged tensors are split into per-chunk TILES (Tile tracks
    dependencies at whole-tile granularity, so chunked tiles are what
    lets the main loop start as soon as chunk 0 is ready).
  - Critical path (K + Q chunk 0) via HWDGE + ACT/DVE fp16 casts; V and
    Q chunks 1-3 stream via SWDGE cast-DMAs (Q gated behind the
    critical-path cast so they can't starve it).
  - QT/KT via merged SBUF->SBUF XBAR transpose-DMAs (3D-out = chunked
    128-partition transposes; K is cast column-duplicated so K^T lands
    replicated on both partition halves).
  - ONE flat 64-step software pipeline over (qb, kt): MM1 two concurrent
    row-group matmuls (contract=64) -> per-head single-bank score tiles
    (3-deep rotation); exp split ACT (true, h0) / DVE (Schraudolph
    int16-bitcast, h1); MM2 (V_aug = [V | ones | 0-pad], col 64 =
    denominator) trails by 4 steps, crossing qb boundaries seamlessly.
    PSUM: 6 score banks + 2 out banks.
  - Drain per (qb, h): PSUM->SBUF fp16 copy (ACT h0 / DVE h1), merged
    XBAR transpose [80,512]->[128,4,80] (h0 Sync ring, h1 Scalar ring,
    concurrent), DVE reciprocal + normalize, un-permuting DMA out
    (final-qb DMAs split across rings). One stage per pipeline step.
  - Two PE warmup phases keep HAM at K=8/8 through the staging window.
"""

import numpy as np

import concourse.bass as bass
import concourse.bacc as bacc
import concourse.mybir as mybir
import concourse.tile as tile
from concourse.bass_utils import run_bass_kernel_spmd

F32 = mybir.dt.float32
F16 = mybir.dt.float16
I16 = mybir.dt.int16

B, H, S, D = 2, 8, 2048, 64
HPC = 2            # heads per core
NCORES = 8
QB = 512           # query block (PSUM bank free-dim limit for fp32)
NQB = S // QB      # 4
KT_TILE = 128      # keys per k-tile (PE contract partition limit)
NKT = S // KT_TILE # 16
NC = NKT // NQB    # 4 c-chunks per qb block
MO = 80            # drained rows: 64 V + 1 denom + 15 pad (16 | 80)
VW = 128           # V_aug weight cols
NSTEP = NQB * NKT  # 64 flat pipeline steps
SCALE = 1.0 / np.sqrt(np.float32(D))  # 0.125
C_SHIFT = 2.0      # exp(z - C_SHIFT): bounds p in fp16/int16; cancels in softmax

# Schraudolph fp16 exp: i16 = round(score*A2 + B2); bitcast fp16 ~= exp(score/8 - C)
A_FP16 = 1024.0 / np.log(2.0)              # 2^10 / ln 2
A2 = float(SCALE) * A_FP16                 # folded score scale
B2 = 15.0 * 1024.0 - C_SHIFT * A_FP16      # exponent bias - shift
SKEW = 3           # MM2 trails MM1 by SKEW+1 steps (check precedes append)


def build_nc():
    nc = bacc.Bacc(None)
    Qd = nc.declare_dram_parameter("q", [HPC, S, D], F32, isOutput=False)
    Kd = nc.declare_dram_parameter("k", [S, D], F32, isOutput=False)
    Vd = nc.declare_dram_parameter("v", [S, D], F32, isOutput=False)
    Od = nc.declare_dram_parameter("o", [HPC, S, D], F32, isOutput=True)

    with tile.TileContext(nc) as tc:
        with (
            tc.tile_pool(name="const", bufs=1) as constp,
            tc.tile_pool(name="stage", bufs=1) as stp,
            tc.tile_pool(name="qk", bufs=1) as qkp,
            tc.tile_pool(name="vt", bufs=1) as vp,
            tc.tile_pool(name="pt", bufs=6) as ptp,
            tc.tile_pool(name="otF", bufs=2) as otFp,
            tc.tile_pool(name="otT", bufs=4) as otTp,
            tc.tile_pool(name="rec", bufs=4) as recp,
            tc.tile_pool(name="outsb", bufs=4) as outp,
            tc.tile_pool(name="psS", bufs=3, space="PSUM") as psSp,
            tc.tile_pool(name="psO", bufs=1, space="PSUM") as psOp,
        ):
            # Prime the exp table load so the ~2.7us ACT_TABLE_LOAD overlaps
            # the input DMA phase instead of stalling the first real exp.
            dummy = constp.tile([128, 64], F16)
            nc.vector.memset(dummy[:], 0.0)
            dummy32 = constp.tile([128, 16], F32)
            nc.vector.memset(dummy32[:], 0.0)
            nc.scalar.activation(
                dummy32[:], dummy32[:], mybir.ActivationFunctionType.Exp
            )

            # per-partition bias AP for the ACT exp (const-AP registry only
            # carries pre-registered values)
            bias_ap = constp.tile([128, 1], F32)
            nc.vector.memset(bias_ap[:], -float(C_SHIFT))

            # warmup output scribbles into the shared out accumulator
            # (cleared by the start=True MM2 later)
            ps_o = [psOp.tile([128, QB], F32, name=f"psO{h}") for h in range(HPC)]

            # PE warmup phase 1: unblocked, runs as soon as the PE is free.
            for _ in range(40):
                nc.tensor.matmul(
                    ps_o[0][0:64, 0:64],
                    lhsT=dummy[:, 0:64],
                    rhs=dummy[:],
                    start=True,
                    stop=True,
                )

            # ---- input staging (all per-c-chunk tiles) ----
            # Critical path: K alone on the Scalar HWDGE ring, Q chunk 0 on
            # the Sync ring ahead of the XBARs that consume it (waiting HWDGE
            # dispatches head-of-line-block their host queue, so nothing
            # compute-bearing may share a ring with long-waiting DMAs).
            Qh = [
                stp.tile([128, NC, HPC, D], F16, name=f"Qh{c}") for c in range(NQB)
            ]
            Qn = stp.tile([128, NC, HPC, D], F32, name="Qn")
            for h in range(HPC):
                nc.sync.dma_start(
                    out=Qn[:, :, h, :],
                    in_=Qd.ap()[h].rearrange("(p c) d -> p c d", p=128)[:, 0:NC, :],
                )
            Kn = stp.tile([128, NKT, D], F32, name="Kn")
            nc.scalar.dma_start(
                out=Kn[:], in_=Kd.ap().rearrange("(p c) d -> p c d", p=128)
            )

            # V_aug chunks [128k, NC, 128] fp16: cols 0-63 = V (same permuted
            # key order as KT), col 64 = 1.0 (denominator), cols 65-127 = 0.
            Vt = [vp.tile([128, NC, VW], F16, name=f"Vt{c}") for c in range(NQB)]
            for c in range(NQB):
                nc.vector.memset(Vt[c][:, :, D + 1 : VW], 0.0)
                nc.vector.memset(Vt[c][:, :, D : D + 1], 1.0)
                nc.gpsimd.dma_start(
                    out=Vt[c][:, :, 0:D],
                    in_=Vd.ap().rearrange("(p c) d -> p c d", p=128)[
                        :, c * NC : (c + 1) * NC, :
                    ],
                )
            for c in range(1, NQB):
                cs = slice(c * NC, (c + 1) * NC)
                for h in range(HPC):
                    nc.gpsimd.dma_start(
                        out=Qh[c][:, :, h, :],
                        in_=Qd.ap()[h].rearrange("(p c) d -> p c d", p=128)[
                            :, cs, :
                        ],
                    )

            # K cast duplicated into both 64-col halves (the XBAR consumes
            # 128-wide input column tiles; the duplicate lands K^T on both
            # partition halves of KT). Halves split across DVE and ACT;
            # Q chunk 0 cast on ACT.
            Kh = [
                stp.tile([128, NC, 2, D], F16, name=f"Kh{c}") for c in range(NQB)
            ]
            for c in range(NQB):
                cs = slice(c * NC, (c + 1) * NC)
                nc.vector.tensor_copy(Kh[c][:, :, 0, :], Kn[:, cs, :])
                nc.scalar.copy(Kh[c][:, :, 1, :], Kn[:, cs, :])
            nc.scalar.copy(Qh[0][:], Qn[:])

            # PE warmup phase 2: gated on the K load so it bridges the
            # staging window right up to the main loop.
            for _ in range(30):
                nc.tensor.matmul(
                    ps_o[0][0:16, 0:64],
                    lhsT=dummy32[:],
                    rhs=Kn[:, 0, :],
                    start=True,
                    stop=True,
                )

            # KT/QT chunks [128, NC, 128] fp16 via merged XBAR transposes on
            # the Sync ring: col (c, p) is key/query s = 16p + c.
            KT = [
                qkp.tile([128, NC, 128], F16, name=f"KT{c}") for c in range(NQB)
            ]
            QT = [
                qkp.tile([128, NC, 128], F16, name=f"QT{c}") for c in range(NQB)
            ]
            for c in range(NQB):
                nc.sync.dma_start(
                    out=KT[c][:],
                    in_=Kh[c][:].rearrange("p c r d -> p (c r d)"),
                    transpose=True,
                )
                nc.sync.dma_start(
                    out=QT[c][:],
                    in_=Qh[c][:].rearrange("p c h d -> p (c h d)"),
                    transpose=True,
                )

            def QTs(h, qb):
                # [64, 512] moving operand for head h, query block qb
                return QT[qb][64 * h : 64 * (h + 1), :, :].rearrange(
                    "p c s -> p (c s)"
                )

            def KTs(h, kt):
                return KT[kt // NC][64 * h : 64 * (h + 1), kt % NC, :]

            # ---- drains, emitted one stage per pipeline step ----
            def drain_stages(qb, ps_o):
                st = {"otF": [None, None], "otT": [None, None], "rec": [None, None],
                      "out": [None, None]}
                ring = [nc.sync, nc.sync]  # Sync ring only (no compute queue)
                last = qb == NQB - 1

                def s_copy(h):
                    def f():
                        st["otF"][h] = otFp.tile([MO, QB], F16, name=f"otF{h}")
                        if h == 0:
                            nc.scalar.copy(st["otF"][h][:], ps_o[h][0:MO, :])
                        else:
                            nc.vector.tensor_copy(st["otF"][h][:], ps_o[h][0:MO, :])
                    return f

                def s_transp(h):
                    def f():
                        st["otT"][h] = otTp.tile(
                            [128, NC, MO], F16, name=f"otT{h}"
                        )
                        ring[h].dma_start(
                            out=st["otT"][h][:], in_=st["otF"][h][:], transpose=True
                        )
                    return f

                def s_recip(h):
                    def f():
                        otT = st["otT"][h]
                        rec = recp.tile([128, NC, 1], F32)
                        nc.vector.reciprocal(rec[:], otT[:, :, D : D + 1])
                        st["rec"][h] = rec
                        st["out"][h] = outp.tile([128, NC, D], F32, name=f"osb{h}")
                        for j in range(NC // 2):
                            nc.vector.tensor_scalar_mul(
                                st["out"][h][:, j, :], otT[:, j, 0:D], rec[:, j, :]
                            )
                    return f

                def s_norm(h):
                    def f():
                        otT, rec, outsb = st["otT"][h], st["rec"][h], st["out"][h]
                        oap = Od.ap()[h].rearrange("(p c) d -> p c d", p=128)
                        for j in range(NC // 2, NC):
                            nc.vector.tensor_scalar_mul(
                                outsb[:, j, :], otT[:, j, 0:D], rec[:, j, :]
                            )
                        if last:
                            # tail: j-paired DMAs across both rings so the
                            # exposed final transfers overlap
                            for j2 in range(2):
                                ring[j2 ^ h].dma_start(
                                    out=oap[
                                        :, qb * NC + 2 * j2 : qb * NC + 2 * j2 + 2, :
                                    ],
                                    in_=outsb[:, 2 * j2 : 2 * j2 + 2, :],
                                )
                        else:
                            ring[h].dma_start(
                                out=oap[:, qb * NC : (qb + 1) * NC, :],
                                in_=outsb[:],
                            )
                    return f

                return [
                    s_copy(0), s_copy(1), s_transp(0), s_transp(1),
                    s_recip(0), s_norm(0), s_recip(1), s_norm(1),
                ]

            # ---- ONE flat software-pipelined loop over all 64 (qb, kt) ----
            pend = []           # (ps_o_pair, qb, kt, pt0, pt1) awaiting MM2
            pending_drain = []  # drain stages of the qb that just finished
            ps_o_cur = ps_o
            for s in range(NSTEP + SKEW + 1):
                if s < NSTEP:
                    qb, kt = divmod(s, NKT)
                    if kt == 0 and qb > 0:
                        ps_o_cur = [
                            psOp.tile([128, QB], F32, name=f"psO{h}")
                            for h in range(HPC)
                        ]
                    ps_s = [
                        psSp.tile([128, QB], F32, name=f"psS{h}")
                        for h in range(HPC)
                    ]
                    for h in range(HPC):
                        nc.tensor.matmul(
                            ps_s[h][:],
                            lhsT=KTs(h, kt),
                            rhs=QTs(h, qb),
                            start=True,
                            stop=True,
                        )
                if len(pend) > (SKEW if s < NSTEP else 0):
                    po, pqb, pkt, p0, p1 = pend.pop(0)
                    for h, rhs in ((0, p0[:]), (1, p1[:].bitcast(F16))):
                        nc.tensor.matmul(
                            po[h][:],
                            lhsT=Vt[pkt // NC][:, pkt % NC, :],
                            rhs=rhs,
                            start=(pkt == 0),
                            stop=(pkt == NKT - 1),
                        )
                    if pkt == NKT - 1:
                        assert not pending_drain
                        pending_drain = drain_stages(pqb, po)
                if pending_drain:
                    pending_drain.pop(0)()
                if s < NSTEP:
                    # exp: ACT (true) for h0, DVE (Schraudolph) for h1
                    pt0 = ptp.tile([128, QB], F16, name="pt0")
                    nc.scalar.activation(
                        pt0[:],
                        ps_s[0][:],
                        mybir.ActivationFunctionType.Exp,
                        scale=float(SCALE),
                        bias=bias_ap[:],
                    )
                    pt1 = ptp.tile([128, QB], I16, name="pt1")
                    nc.vector.tensor_scalar(
                        pt1[:],
                        ps_s[1][:],
                        float(A2),
                        float(B2),
                        op0=mybir.AluOpType.mult,
                        op1=mybir.AluOpType.add,
                    )
                    pend.append((ps_o_cur, qb, kt, pt0, pt1))
            assert not pend
            for f in pending_drain:
                f()
    nc.compile()
    return nc


_CACHED = {}


def _get_nc():
    if "nc" not in _CACHED:
        _CACHED["nc"] = build_nc()
    return _CACHED["nc"]


def _shard(Q, K, V):
    in_maps = []
    for c in range(NCORES):
        b = c // 4
        h0 = (c % 4) * HPC
        in_maps.append(
            {
                "q": np.ascontiguousarray(np.asarray(Q, np.float32)[b, h0 : h0 + HPC]),
                "k": np.ascontiguousarray(np.asarray(K, np.float32)[b, 0]),
                "v": np.ascontiguousarray(np.asarray(V, np.float32)[b, 0]),
            }
        )
    return in_maps


def kernel(Q, K, V, trace=False):
    nc = _get_nc()
    res = run_bass_kernel_spmd(nc, _shard(Q, K, V), list(range(NCORES)), trace=trace)
    _CACHED["last_result"] = res
    O = np.empty((B, H, S, D), np.float32)
    for c, r in enumerate(res.results):
        b = c // 4
        h0 = (c % 4) * HPC
        O[b, h0 : h0 + HPC] = r["o"]
    return O


# revision 39
# speedup vs baseline: 1.2386x; 1.1495x over previous
"""MQA attention kernel for Trainium2, 8-core SPMD (v11).

Problem: Q [2, 8, 2048, 64] fp32, K/V [2, 1, 2048, 64] fp32 (shared head).
out[b,h,q,:] = softmax(Q[b,h,q,:] @ K[b,0]^T / 8) @ V[b,0].

Sharding: 16 (b,h) pairs over 8 cores -> core c handles b = c//4,
heads 2*(c%4), 2*(c%4)+1 (both heads share one K/V slice).

Design notes:
  - "Permuted-s" staging: inputs land as Xh[p, c, ...] = X[s=16p+c]
    (contiguous 4KB descriptors; softmax doesn't care about key/query
    order, V uses the same key order; only the output DMA un-permutes).
  - All staged tensors are split into per-chunk TILES (Tile tracks
    dependencies at whole-tile granularity, so chunked tiles are what
    lets the main loop start as soon as chunk 0 is ready).
  - Critical path (K + Q chunk 0) via HWDGE + ACT/DVE fp16 casts; V and
    Q chunks 1-3 stream via SWDGE cast-DMAs (Q gated behind the
    critical-path cast so they can't starve it).
  - QT/KT via merged SBUF->SBUF XBAR transpose-DMAs (3D-out = chunked
    128-partition transposes; K is cast column-duplicated so K^T lands
    replicated on both partition halves).
  - ONE flat 64-step software pipeline over (qb, kt): MM1 two concurrent
    row-group matmuls (contract=64) -> per-head single-bank score tiles
    (3-deep rotation); exp split ACT (true, h0) / DVE (Schraudolph
    int16-bitcast, h1); MM2 (V_aug = [V | ones | 0-pad], col 64 =
    denominator) trails by 4 steps, crossing qb boundaries seamlessly.
    PSUM: 6 score banks + 2 out banks.
  - Drain per (qb, h): PSUM->SBUF fp16 copy (ACT h0 / DVE h1), merged
    XBAR transpose [80,512]->[128,4,80] (h0 Sync ring, h1 Scalar ring,
    concurrent), DVE reciprocal + normalize, un-permuting DMA out
    (final-qb DMAs split across rings). One stage per pipeline step.
  - Two PE warmup phases keep HAM at K=8/8 through the staging window.
"""

import numpy as np

import concourse.bass as bass
import concourse.bacc as bacc
import concourse.mybir as mybir
import concourse.tile as tile
from concourse.bass_utils import run_bass_kernel_spmd

F32 = mybir.dt.float32
F16 = mybir.dt.float16
I16 = mybir.dt.int16

B, H, S, D = 2, 8, 2048, 64
HPC = 2            # heads per core
NCORES = 8
QB = 512           # query block (PSUM bank free-dim limit for fp32)
NQB = S // QB      # 4
KT_TILE = 128      # keys per k-tile (PE contract partition limit)
NKT = S // KT_TILE # 16
NC = NKT // NQB    # 4 c-chunks per qb block
MO = 80            # drained rows: 64 V + 1 denom + 15 pad (16 | 80)
VW = 128           # V_aug weight cols
NSTEP = NQB * NKT  # 64 flat pipeline steps
SCALE = 1.0 / np.sqrt(np.float32(D))  # 0.125
C_SHIFT = 2.0      # exp(z - C_SHIFT): bounds p in fp16/int16; cancels in softmax

# Schraudolph fp16 exp: i16 = round(score*A2 + B2); bitcast fp16 ~= exp(score/8 - C)
A_FP16 = 1024.0 / np.log(2.0)              # 2^10 / ln 2
A2 = float(SCALE) * A_FP16                 # folded score scale
B2 = 15.0 * 1024.0 - C_SHIFT * A_FP16      # exponent bias - shift
SKEW = 3           # MM2 trails MM1 by SKEW+1 steps (check precedes append)


def build_nc():
    nc = bacc.Bacc(None)
    Qd = nc.declare_dram_parameter("q", [HPC, S, D], F32, isOutput=False)
    Kd = nc.declare_dram_parameter("k", [S, D], F32, isOutput=False)
    Vd = nc.declare_dram_parameter("v", [S, D], F32, isOutput=False)
    Od = nc.declare_dram_parameter("o", [HPC, S, D], F32, isOutput=True)

    with tile.TileContext(nc) as tc:
        with (
            tc.tile_pool(name="const", bufs=1) as constp,
            tc.tile_pool(name="stage", bufs=1) as stp,
            tc.tile_pool(name="qk", bufs=1) as qkp,
            tc.tile_pool(name="vt", bufs=1) as vp,
            tc.tile_pool(name="pt", bufs=6) as ptp,
            tc.tile_pool(name="otF", bufs=2) as otFp,
            tc.tile_pool(name="otT", bufs=4) as otTp,
            tc.tile_pool(name="rec", bufs=4) as recp,
            tc.tile_pool(name="outsb", bufs=4) as outp,
            tc.tile_pool(name="psS", bufs=3, space="PSUM") as psSp,
            tc.tile_pool(name="psO", bufs=1, space="PSUM") as psOp,
        ):
            # Prime the exp table load so the ~2.7us ACT_TABLE_LOAD overlaps
            # the input DMA phase instead of stalling the first real exp.
            dummy = constp.tile([128, 64], F16)
            nc.vector.memset(dummy[:], 0.0)
            dummy32 = constp.tile([128, 16], F32)
            nc.vector.memset(dummy32[:], 0.0)
            nc.scalar.activation(
                dummy32[:], dummy32[:], mybir.ActivationFunctionType.Exp
            )

            # per-partition bias AP for the ACT exp (const-AP registry only
            # carries pre-registered values)
            bias_ap = constp.tile([128, 1], F32)
            nc.vector.memset(bias_ap[:], -float(C_SHIFT))

            # warmup output scribbles into the shared out accumulator
            # (cleared by the start=True MM2 later)
            ps_o = [psOp.tile([128, QB], F32, name=f"psO{h}") for h in range(HPC)]

            # PE warmup phase 1: unblocked, runs as soon as the PE is free.
            for _ in range(40):
                nc.tensor.matmul(
                    ps_o[0][0:64, 0:64],
                    lhsT=dummy[:, 0:64],
                    rhs=dummy[:],
                    start=True,
                    stop=True,
                )

            # ---- input staging (all per-c-chunk tiles, all HWDGE) ----
            # XBAR transposes serialize against outstanding DMA streams, so
            # everything loads up-front via small HWDGE chunk-DMAs (32-128
            # descriptors each: the ring never stalls, descriptors spread
            # across the DMA engines). fp16 casts chase chunks on ACT+DVE.
            Kn = stp.tile([128, NKT, D], F32, name="Kn")
            nc.scalar.dma_start(
                out=Kn[:], in_=Kd.ap().rearrange("(p c) d -> p c d", p=128)
            )
            Qn = [
                stp.tile([128, NC, HPC, D], F32, name=f"Qn{c}") for c in range(NQB)
            ]
            for c in range(NQB):
                cs = slice(c * NC, (c + 1) * NC)
                for h in range(HPC):
                    nc.sync.dma_start(
                        out=Qn[c][:, :, h, :],
                        in_=Qd.ap()[h].rearrange("(p c) d -> p c d", p=128)[
                            :, cs, :
                        ],
                    )
            Vn = stp.tile([128, NKT, D], F32, name="Vn")
            nc.scalar.dma_start(
                out=Vn[:], in_=Vd.ap().rearrange("(p c) d -> p c d", p=128)
            )

            # K cast duplicated into both 64-col halves (the XBAR consumes
            # 128-wide input column tiles; the duplicate lands K^T on both
            # partition halves of KT). DVE: K-r0 + V; ACT: K-r1 + Q.
            Kh = [
                stp.tile([128, NC, 2, D], F16, name=f"Kh{c}") for c in range(NQB)
            ]
            Qh = [
                stp.tile([128, NC, HPC, D], F16, name=f"Qh{c}") for c in range(NQB)
            ]
            # V_aug chunks [128k, NC, 128] fp16: cols 0-63 = V (same permuted
            # key order as KT), col 64 = 1.0 (denominator), cols 65-127 = 0.
            Vt = [vp.tile([128, NC, VW], F16, name=f"Vt{c}") for c in range(NQB)]
            for c in range(NQB):
                cs = slice(c * NC, (c + 1) * NC)
                nc.vector.tensor_copy(Kh[c][:, :, 0, :], Kn[:, cs, :])
                nc.scalar.copy(Kh[c][:, :, 1, :], Kn[:, cs, :])
                nc.scalar.copy(Qh[c][:], Qn[c][:])
                nc.vector.memset(Vt[c][:, :, D + 1 : VW], 0.0)
                nc.vector.memset(Vt[c][:, :, D : D + 1], 1.0)
                nc.vector.tensor_copy(Vt[c][:, :, 0:D], Vn[:, cs, :])

            # PE warmup phase 2: gated on the K load so it bridges the
            # staging window right up to the main loop.
            for _ in range(30):
                nc.tensor.matmul(
                    ps_o[0][0:16, 0:64],
                    lhsT=dummy32[:],
                    rhs=Kn[:, 0, :],
                    start=True,
                    stop=True,
                )

            # KT/QT chunks [128, NC, 128] fp16 via merged XBAR transposes on
            # the Sync ring: col (c, p) is key/query s = 16p + c.
            KT = [
                qkp.tile([128, NC, 128], F16, name=f"KT{c}") for c in range(NQB)
            ]
            QT = [
                qkp.tile([128, NC, 128], F16, name=f"QT{c}") for c in range(NQB)
            ]
            for c in range(NQB):
                nc.sync.dma_start(
                    out=KT[c][:],
                    in_=Kh[c][:].rearrange("p c r d -> p (c r d)"),
                    transpose=True,
                )
                nc.sync.dma_start(
                    out=QT[c][:],
                    in_=Qh[c][:].rearrange("p c h d -> p (c h d)"),
                    transpose=True,
                )

            def QTs(h, qb):
                # [64, 512] moving operand for head h, query block qb
                return QT[qb][64 * h : 64 * (h + 1), :, :].rearrange(
                    "p c s -> p (c s)"
                )

            def KTs(h, kt):
                return KT[kt // NC][64 * h : 64 * (h + 1), kt % NC, :]

            # ---- drains, emitted one stage per pipeline step ----
            def drain_stages(qb, ps_o):
                st = {"otF": [None, None], "otT": [None, None], "rec": [None, None],
                      "out": [None, None]}
                ring = [nc.sync, nc.sync]  # Sync ring only (no compute queue)
                last = qb == NQB - 1

                def s_copy(h):
                    def f():
                        st["otF"][h] = otFp.tile([MO, QB], F16, name=f"otF{h}")
                        if h == 0:
                            nc.scalar.copy(st["otF"][h][:], ps_o[h][0:MO, :])
                        else:
                            nc.vector.tensor_copy(st["otF"][h][:], ps_o[h][0:MO, :])
                    return f

                def s_transp(h):
                    def f():
                        st["otT"][h] = otTp.tile(
                            [128, NC, MO], F16, name=f"otT{h}"
                        )
                        ring[h].dma_start(
                            out=st["otT"][h][:], in_=st["otF"][h][:], transpose=True
                        )
                    return f

                def s_recip(h):
                    def f():
                        otT = st["otT"][h]
                        rec = recp.tile([128, NC, 1], F32)
                        nc.vector.reciprocal(rec[:], otT[:, :, D : D + 1])
                        st["rec"][h] = rec
                        st["out"][h] = outp.tile([128, NC, D], F32, name=f"osb{h}")
                        for j in range(NC // 2):
                            nc.vector.tensor_scalar_mul(
                                st["out"][h][:, j, :], otT[:, j, 0:D], rec[:, j, :]
                            )
                    return f

                def s_norm(h):
                    def f():
                        otT, rec, outsb = st["otT"][h], st["rec"][h], st["out"][h]
                        oap = Od.ap()[h].rearrange("(p c) d -> p c d", p=128)
                        for j in range(NC // 2, NC):
                            nc.vector.tensor_scalar_mul(
                                outsb[:, j, :], otT[:, j, 0:D], rec[:, j, :]
                            )
                        if last:
                            # tail: j-paired DMAs across both rings so the
                            # exposed final transfers overlap
                            for j2 in range(2):
                                ring[j2 ^ h].dma_start(
                                    out=oap[
                                        :, qb * NC + 2 * j2 : qb * NC + 2 * j2 + 2, :
                                    ],
                                    in_=outsb[:, 2 * j2 : 2 * j2 + 2, :],
                                )
                        else:
                            ring[h].dma_start(
                                out=oap[:, qb * NC : (qb + 1) * NC, :],
                                in_=outsb[:],
                            )
                    return f

                return [
                    s_copy(0), s_copy(1), s_transp(0), s_transp(1),
                    s_recip(0), s_norm(0), s_recip(1), s_norm(1),
                ]

            # ---- ONE flat software-pipelined loop over all 64 (qb, kt) ----
            pend = []           # (ps_o_pair, qb, kt, pt0, pt1) awaiting MM2
            pending_drain = []  # drain stages of the qb that just finished
            ps_o_cur = ps_o
            for s in range(NSTEP + SKEW + 1):
                if s < NSTEP:
                    qb, kt = divmod(s, NKT)
                    if kt == 0 and qb > 0:
                        ps_o_cur = [
                            psOp.tile([128, QB], F32, name=f"psO{h}")
                            for h in range(HPC)
                        ]
                    ps_s = [
                        psSp.tile([128, QB], F32, name=f"psS{h}")
                        for h in range(HPC)
                    ]
                    for h in range(HPC):
                        nc.tensor.matmul(
                            ps_s[h][:],
                            lhsT=KTs(h, kt),
                            rhs=QTs(h, qb),
                            start=True,
                            stop=True,
                        )
                if len(pend) > (SKEW if s < NSTEP else 0):
                    po, pqb, pkt, p0, p1 = pend.pop(0)
                    for h, rhs in ((0, p0[:]), (1, p1[:].bitcast(F16))):
                        nc.tensor.matmul(
                            po[h][:],
                            lhsT=Vt[pkt // NC][:, pkt % NC, :],
                            rhs=rhs,
                            start=(pkt == 0),
                            stop=(pkt == NKT - 1),
                        )
                    if pkt == NKT - 1:
                        assert not pending_drain
                        pending_drain = drain_stages(pqb, po)
                if pending_drain:
                    pending_drain.pop(0)()
                if s < NSTEP:
                    # exp: ACT (true) for h0, DVE (Schraudolph) for h1
                    pt0 = ptp.tile([128, QB], F16, name="pt0")
                    nc.scalar.activation(
                        pt0[:],
                        ps_s[0][:],
                        mybir.ActivationFunctionType.Exp,
                        scale=float(SCALE),
                        bias=bias_ap[:],
                    )
                    pt1 = ptp.tile([128, QB], I16, name="pt1")
                    nc.vector.tensor_scalar(
                        pt1[:],
                        ps_s[1][:],
                        float(A2),
                        float(B2),
                        op0=mybir.AluOpType.mult,
                        op1=mybir.AluOpType.add,
                    )
                    pend.append((ps_o_cur, qb, kt, pt0, pt1))
            assert not pend
            for f in pending_drain:
                f()
    nc.compile()
    return nc


_CACHED = {}


def _get_nc():
    if "nc" not in _CACHED:
        _CACHED["nc"] = build_nc()
    return _CACHED["nc"]


def _shard(Q, K, V):
    in_maps = []
    for c in range(NCORES):
        b = c // 4
        h0 = (c % 4) * HPC
        in_maps.append(
            {
                "q": np.ascontiguousarray(np.asarray(Q, np.float32)[b, h0 : h0 + HPC]),
                "k": np.ascontiguousarray(np.asarray(K, np.float32)[b, 0]),
                "v": np.ascontiguousarray(np.asarray(V, np.float32)[b, 0]),
            }
        )
    return in_maps


def kernel(Q, K, V, trace=False):
    nc = _get_nc()
    res = run_bass_kernel_spmd(nc, _shard(Q, K, V), list(range(NCORES)), trace=trace)
    _CACHED["last_result"] = res
    O = np.empty((B, H, S, D), np.float32)
    for c, r in enumerate(res.results):
        b = c // 4
        h0 = (c % 4) * HPC
        O[b, h0 : h0 + HPC] = r["o"]
    return O
